# revision 1
# baseline (speedup 1.0000x reference)
"""TGCN (dense-graph GRU) Trainium2 kernel, 8-core SPMD, no collectives.

Math (per reference):
  xh_t = relu(x_t @ fc_w + fc_b)                    [N, H]
  S_t  = adj @ xh_t                                 (assoc: adj@(xh@W) = (adj@xh)@W)
  z_t  = sigmoid(S_t @ Mz + h @ Lz_bot + bz)        Mz = Wz @ Lz_top (host-folded)
  r_t  = sigmoid(S_t @ Mr + h @ Lr_bot + br)
  ht_t = tanh   (S_t @ Mh + (h*r) @ Lh_bot + bh)
  h    = z*h + (1-z)*ht = ht + z*(h - ht)

Sharding: row-partition adj across 8 cores (512 nodes each). The GRU cell is
row-local, so each core runs the whole time loop on its shard independently.
x is replicated (each core redundantly computes xh for all nodes — cheaper
than an all-gather through DRAM bounce buffers).

Layout: everything on-chip is feature-major ("transposed"): S_t.T, z.T, h.T
are [64 feat, 512 nodes]. This makes every matmul operand natural-layout
(weights [K, M] as stored, x host-transposed to [T, F, N]) — zero on-chip
transposes. Time steps are processed in pairs so the big adj matmul has
M=128 (full PE array): lhsT = [xh_t | xh_t+1] tiles, rhs = adjT tiles.

Each GRU gate is ONE K=128 matmul: stationary [M*; L*_bot] stacked on the
contraction dim, rhs a [S_t.T; h.T] concat tile whose bottom half IS the
recurrent state (the combine writes h directly into the next concat buffer,
rotating over 4 buffers so per-pair S refreshes never serialize with the
chain). ACT's ability to read/write at shifted partition bases glues the
[0:64]/[64:128] halves together; matmul operands at base 64 crash TRN2, so
all matmul rhs/lhsT tiles start at partition 0 or are full 128-partition.

Dtypes: all matmuls bf16 (1 cyc/row; fp32 is 4, float32r requires
producer-side rounding walrus rejects); h state bf16; PSUM f32.
Measured: ~366-373us HW exec, rel err ~4.5e-3 vs the f32 reference.
"""

import os
import sys

sys.path.insert(0, "/opt/trn_rl_repo")

import numpy as np
import ml_dtypes

T, N, F_IN, H1, F_OUT = 48, 4096, 64, 64, 64
NCORES = 8
NS = N // NCORES          # nodes per core = 512
PAIRS = T // 2            # 24
KT = N // 128             # 32 contraction tiles for the adj matmul

_cache = {}


def _build():
    import concourse.bass as bass
    import concourse.mybir as mybir
    import concourse.tile as tile
    from concourse import bacc

    f32 = mybir.dt.float32
    f32r = mybir.dt.float32r
    bf16 = mybir.dt.bfloat16
    AF = mybir.ActivationFunctionType

    nc = bacc.Bacc(
        "TRN2",
        target_bir_lowering=False,
        debug=False,
        enable_asserts=False,
        num_devices=NCORES,
    )

    # DRAM parameters (per-core shapes)
    adjT_d = nc.dram_tensor("adjT", [128, KT, NS], bf16, kind="ExternalInput").ap()
    xT_d = nc.dram_tensor("xT", [PAIRS, F_IN, 2, N], bf16, kind="ExternalInput").ap()
    fcw_d = nc.dram_tensor("fcw", [F_IN, H1], bf16, kind="ExternalInput").ap()
    wzr_d = nc.dram_tensor("wzr", [128, 128], bf16, kind="ExternalInput").ap()
    wh_d = nc.dram_tensor("wh", [128, F_OUT], bf16, kind="ExternalInput").ap()
    bz_d = nc.dram_tensor("bz", [F_OUT, 1], f32, kind="ExternalInput").ap()
    bzn_d = nc.dram_tensor("bzn", [F_OUT, 1], f32, kind="ExternalInput").ap()
    br_d = nc.dram_tensor("br", [F_OUT, 1], f32, kind="ExternalInput").ap()
    bh_d = nc.dram_tensor("bh", [F_OUT, 1], f32, kind="ExternalInput").ap()
    out_d = nc.dram_tensor("out", [F_OUT, NS], f32, kind="ExternalOutput").ap()

    with tile.TileContext(nc) as tc:
        with (
            tc.tile_pool(name="const", bufs=1) as constp,
            tc.tile_pool(name="state", bufs=1) as statep,
            tc.tile_pool(name="xt", bufs=2) as xtp,
            tc.tile_pool(name="xh", bufs=2) as xhp,
            tc.tile_pool(name="gw", bufs=3) as gwp,
            tc.tile_pool(name="psx", bufs=2, space="PSUM") as psxp,
            tc.tile_pool(name="pss", bufs=3, space="PSUM") as pssp,
            tc.tile_pool(name="pszr", bufs=2, space="PSUM") as pszrp,
            tc.tile_pool(name="psh", bufs=1, space="PSUM") as pshp,
        ):
            # ---- constants ----
            # fcw first so pair 0 can start immediately; adjT is host-pre-tiled
            # to SBUF layout (contiguous 32KB per partition -> cheap DMA)
            fcw_sb = constp.tile([F_IN, H1], bf16)
            nc.sync.dma_start(out=fcw_sb[:], in_=fcw_d[:])
            adjT_sb = constp.tile([128, KT, NS], bf16)
            for q, eng in enumerate((nc.sync, nc.gpsimd, nc.gpsimd, nc.gpsimd)):
                eng.dma_start(
                    out=adjT_sb[:, q * 8 : (q + 1) * 8, :],
                    in_=adjT_d[:, q * 8 : (q + 1) * 8, :],
                )
            wzr_sb = constp.tile([128, 128], bf16)
            wh_sb = constp.tile([128, F_OUT], bf16)
            bz_sb = constp.tile([F_OUT, 1], f32)
            bzn_sb = constp.tile([F_OUT, 1], f32)
            br_sb = constp.tile([F_OUT, 1], f32)
            bh_sb = constp.tile([F_OUT, 1], f32)
            for dst, src in (
                (wzr_sb, wzr_d), (wh_sb, wh_d),
                (bz_sb, bz_d), (bzn_sb, bzn_d), (br_sb, br_d), (bh_sb, bh_d),
            ):
                nc.gpsimd.dma_start(out=dst[:], in_=src[:])

            # ---- state ----
            # Concat rhs tiles for the K=128 gate matmuls: rows 0-63 carry
            # S_t.T (refreshed per step, off-chain), rows 64-127 carry the
            # recurrent state: h.T in CzS_*, (h*r).T in ChS_*. h ping-pongs
            # between the two CzS buffers (the combine writes the other one).
            CzS = []
            ChS = []
            for i in range(4):
                czsi = statep.tile([128, NS], bf16, tag=f"CzS{i}")
                chsi = statep.tile([128, NS], bf16, tag=f"ChS{i}")
                CzS.append(czsi)
                ChS.append(chsi)
            nc.vector.memset(CzS[0][:], 0.0)

            S_prev = None  # S-pair tile of the previous pair

            def emit_xh_groups(xt, xh, groups):
                # xh-pair matmuls: out[128 nodes, 64] = xT_slice.T @ fcw.
                # All operands at partition base 0 (base-64 matmul operands
                # crash the exec unit on TRN2). Steps t / t+1 select the free
                # axis of xt. 8 matmuls fill one PSUM bank laid out to match
                # xh's [node-tile, t|t+1] column order.
                for g in groups:
                    ps = psxp.tile([128, 512], mybir.dt.float32)
                    for j in range(4):
                        k = 4 * g + j
                        for s in (0, 1):
                            nc.tensor.matmul(
                                ps[:, j * 128 + s * 64 : j * 128 + (s + 1) * 64],
                                lhsT=xt[:, s, k * 128 : (k + 1) * 128],
                                rhs=fcw_sb[:],
                                start=True, stop=True,
                            )
                    nc.any.tensor_relu(
                        xh[:, 4 * g : 4 * (g + 1), :].rearrange("p a b -> p (a b)"),
                        ps[:],
                    )

            def emit_gru_front(step):
                # zr matmul + sigmoids + h*r / z*h products for one step.
                cur = CzS[step % 4]
                ch = ChS[step % 4]
                H = cur[64:128, :]

                ps_zr = pszrp.tile([128, NS], mybir.dt.float32, tag="ps_zr")
                nc.tensor.matmul(ps_zr[:], lhsT=wzr_sb[:], rhs=cur[:],
                                 start=True, stop=True)
                # r first: it gates the h-candidate matmul (critical chain);
                # z / (1-z) / z*h all run off-chain in parallel
                R = gwp.tile([128, NS], bf16, tag="R")
                nc.scalar.activation(R[64:128, :], ps_zr[64:128, :],
                                     AF.Sigmoid, bias=br_sb[:])
                Z = gwp.tile([128, NS], bf16, tag="Z")
                nc.scalar.activation(Z[64:128, :], ps_zr[0:64, :],
                                     AF.Sigmoid, bias=bz_sb[:])
                ZC = gwp.tile([128, NS], bf16, tag="ZC")
                nc.vector.tensor_scalar(ZC[64:128, :], Z[64:128, :], -1.0, 1.0,
                                        mybir.AluOpType.mult, mybir.AluOpType.add)
                nc.vector.tensor_mul(ch[64:128, :], H, R[64:128, :])
                A1 = gwp.tile([128, NS], bf16, tag="A1")
                nc.vector.tensor_mul(A1[64:128, :], Z[64:128, :], H)
                return ZC, A1

            def emit_gru_back(step, ZC, A1):
                # h-candidate matmul + tanh + combine into the next buffer.
                ch = ChS[step % 4]
                nxt = CzS[(step + 1) % 4]
                ps_h = pshp.tile([F_OUT, NS], mybir.dt.float32)
                nc.tensor.matmul(ps_h[:], lhsT=wh_sb[:], rhs=ch[:],
                                 start=True, stop=True)
                HT = gwp.tile([128, NS], bf16, tag="HT")
                nc.scalar.activation(HT[64:128, :], ps_h[:], AF.Tanh,
                                     bias=bh_sb[:])
                # h_new = z*h + (1-z)*ht -> bottom half of the NEXT buffer
                B1 = gwp.tile([128, NS], bf16, tag="B1")
                nc.vector.tensor_mul(B1[64:128, :], ZC[64:128, :], HT[64:128, :])
                nc.vector.tensor_add(nxt[64:128, :], A1[64:128, :], B1[64:128, :])

            # ---- main loop, software-pipelined: gates of pair p-1 are
            # emitted between the xh/A matmul bursts of pair p so the
            # sequential GRU chain hides under parallel PE work. ----
            for p in range(PAIRS):
                xt = xtp.tile([F_IN, 2, N], bf16)
                # alternate DMA queues: a [64, ...] DMA only drives half the
                # ports, two queues in flight recover full bandwidth
                (nc.scalar if p % 2 == 0 else nc.sync).dma_start(
                    out=xt[:], in_=xT_d[p]
                )
                xh = xhp.tile([128, KT, 128], bf16)

                # previous pair's two GRU steps are interleaved between the
                # xh matmul groups: every chain wait (ACT sigma / DVE mul) is
                # covered by queued PE work, and the chain's ACT/DVE ops stay
                # ahead of the bulk relu/copy traffic in each engine FIFO
                if p >= 1:
                    fr0 = emit_gru_front(2 * p - 2)
                emit_xh_groups(xt, xh, range(0, 2))
                if p >= 1:
                    emit_gru_back(2 * p - 2, *fr0)
                emit_xh_groups(xt, xh, range(2, 4))
                if p >= 1:
                    fr1 = emit_gru_front(2 * p - 1)
                emit_xh_groups(xt, xh, range(4, 6))
                if p >= 1:
                    emit_gru_back(2 * p - 1, *fr1)
                emit_xh_groups(xt, xh, range(6, 8))

                # S-pair matmul: psS[2*64 feat, 512 my-nodes] accumulated
                # over 32 node K-tiles.
                psS = pssp.tile([128, NS], mybir.dt.float32)
                for k in range(KT):
                    nc.tensor.matmul(
                        psS[:],
                        lhsT=xh[:, k, :],
                        rhs=adjT_sb[:, k, :],
                        start=(k == 0), stop=(k == KT - 1),
                    )
                # refresh concat tops for this pair's two steps; the 4-way
                # rotation means these buffers were last read two pairs ago,
                # so the copies run fully off the sequential gate chain
                s0, s1 = (2 * p) % 4, (2 * p + 1) % 4
                nc.scalar.copy(CzS[s0][0:64, :], psS[0:64, :])
                nc.vector.tensor_copy(ChS[s0][0:64, :], psS[0:64, :])
                nc.scalar.copy(CzS[s1][0:64, :], psS[64:128, :])
                nc.vector.tensor_copy(ChS[s1][0:64, :], psS[64:128, :])

            # drain: gates for the last pair
            fr = emit_gru_front(2 * PAIRS - 2)
            emit_gru_back(2 * PAIRS - 2, *fr)
            fr = emit_gru_front(2 * PAIRS - 1)
            emit_gru_back(2 * PAIRS - 1, *fr)

            Hout = statep.tile([F_OUT, NS], f32)
            nc.scalar.copy(Hout[:], CzS[(2 * PAIRS) % 4][64:128, :])
            nc.sync.dma_start(out=out_d[:], in_=Hout[:])

    nc.compile()
    return nc


def _prep_inputs(x, adj, fc_w, Wz, Wr, Wh, Lz, Lr, Lh, bz, br, bh):
    bf16 = ml_dtypes.bfloat16
    f32 = np.float32

    # x [T, N, F] -> [PAIRS, F, step, N] (features on partitions)
    xT = np.ascontiguousarray(
        x.reshape(PAIRS, 2, N, F_IN).transpose(0, 3, 1, 2)
    ).astype(bf16)
    fcw = fc_w.astype(bf16)

    def fold(W, L):
        return (W.astype(np.float64) @ L[:F_OUT].astype(np.float64)).astype(bf16)

    mz, mr, mh = fold(Wz, Lz), fold(Wr, Lr), fold(Wh, Lh)
    mzr = np.concatenate([mz, mr], axis=1)  # [64, 128]: z cols | r cols
    lzr = np.concatenate(
        [Lz[F_OUT:].astype(bf16), Lr[F_OUT:].astype(bf16)], axis=1
    )
    # stacked [K=128] weights: rows 0-63 hit S_t, rows 64-127 hit h / (h*r)
    wzr = np.concatenate([mzr, lzr], axis=0)  # [128, 128]
    wh = np.concatenate(
        [mh, Lh[F_OUT:].astype(bf16)], axis=0
    )  # [128, 64]
    shared = {
        "xT": xT, "fcw": fcw, "wzr": wzr, "wh": wh,
        "bz": bz.reshape(F_OUT, 1).astype(f32),
        "bzn": (-bz).reshape(F_OUT, 1).astype(f32),
        "br": br.reshape(F_OUT, 1).astype(f32),
        "bh": bh.reshape(F_OUT, 1).astype(f32),
    }
    in_maps = []
    for c in range(NCORES):
        m = dict(shared)
        at = adj[c * NS : (c + 1) * NS, :].T  # [N, NS]
        m["adjT"] = np.ascontiguousarray(
            at.reshape(KT, 128, NS).transpose(1, 0, 2)
        ).astype(bf16)
        in_maps.append(m)
    return in_maps


def kernel(x, adj, fc_w, fc_b, Wz, Wr, Wh, Lz, Lr, Lh, bz, br, bh):
    x = np.asarray(x, np.float32)
    adj = np.asarray(adj, np.float32)
    args = [np.asarray(a, np.float32) for a in (fc_w, Wz, Wr, Wh, Lz, Lr, Lh, bz, br, bh)]
    fc_b = np.asarray(fc_b, np.float32)
    if np.any(fc_b != 0.0):
        # fc_b can't fold into the per-partition activation bias (it varies
        # along the free dim); the reference always passes zeros. Pure-numpy
        # fallback keeps kernel() correct for arbitrary inputs.
        return _numpy_ref(x, adj, args[0], fc_b, *args[1:])

    from concourse.bass_utils import run_bass_kernel_spmd

    if "nc" not in _cache:
        _cache["nc"] = _build()
    nc = _cache["nc"]

    in_maps = _prep_inputs(x, adj, *args)
    trace = bool(int(os.environ.get("BASS_KERNEL_TRACE", "0")))
    kwargs = {}
    if trace:
        _install_trace_shim()
        tmpdir = os.environ.get("BASS_KERNEL_TRACE_DIR")
        if tmpdir:
            os.makedirs(tmpdir, exist_ok=True)
            kwargs["tmpdir"] = tmpdir
    res = run_bass_kernel_spmd(
        nc, in_maps, core_ids=list(range(NCORES)), trace=trace, **kwargs
    )
    _cache["last_result"] = res

    out = np.empty((1, N, F_OUT), np.float32)
    for c in range(NCORES):
        out[0, c * NS : (c + 1) * NS, :] = res.results[c]["out"].T
    return out


def _install_trace_shim():
    """Register the NTFF profile hook (this image's antenv lacks axon_hooks)
    and stub out the artifact upload so profiling works offline."""
    import types

    try:
        from antenv import axon_hooks  # noqa: F401
        return
    except ImportError:
        pass
    sys.path.insert(0, "/root/.axon_site")
    from trn_agent_boot.trn_boot import _ntff_profile_via_ctypes

    hook = _ntff_profile_via_ctypes("/opt/axon/libaxon_pjrt.so")
    m = types.ModuleType("antenv.axon_hooks")
    m.get_axon_ntff_profile_hook = lambda: hook
    m.set_axon_ntff_profile_hook = lambda h: None
    sys.modules["antenv.axon_hooks"] = m
    import antenv

    antenv.axon_hooks = m
    from concourse import bass_utils as _bu

    _bu.upload_artifacts = lambda tmpdir: tmpdir


def _numpy_ref(x, adj, fc_w, fc_b, Wz, Wr, Wh, Lz, Lr, Lh, bz, br, bh):
    def sigmoid(v):
        return 1.0 / (1.0 + np.exp(-v))

    xh = np.maximum(x @ fc_w + fc_b, 0.0)
    h = np.zeros((N, F_OUT), np.float32)
    for t in range(T):
        s = adj @ xh[t]
        az, ar, ah = s @ Wz, s @ Wr, s @ Wh
        z = sigmoid(np.concatenate([az, h], -1) @ Lz + bz)
        r = sigmoid(np.concatenate([ar, h], -1) @ Lr + br)
        ht = np.tanh(np.concatenate([ah, h * r], -1) @ Lh + bh)
        h = z * h + (1.0 - z) * ht
    return h[None].astype(np.float32)



# revision 3
# speedup vs baseline: 1.0516x; 1.0516x over previous
"""TGCN (dense-graph GRU) Trainium2 kernel, 8-core SPMD, no collectives.

Math (per reference):
  xh_t = relu(x_t @ fc_w + fc_b)                    [N, H]
  S_t  = adj @ xh_t                                 (assoc: adj@(xh@W) = (adj@xh)@W)
  z_t  = sigmoid(S_t @ Mz + h @ Lz_bot + bz)        Mz = Wz @ Lz_top (host-folded)
  r_t  = sigmoid(S_t @ Mr + h @ Lr_bot + br)
  ht_t = tanh   (S_t @ Mh + (h*r) @ Lh_bot + bh)
  h    = (1-z)*ht + z*h = h + (1-z)*(ht - h)

Sharding: row-partition adj across 8 cores (512 nodes each). The GRU cell is
row-local, so each core runs the whole time loop on its shard independently.
x is replicated (each core redundantly computes xh for all nodes — an
all-gather per step would be latency-bound, ~10us per 64KB shard).

Layout: feature-major on-chip ("transposed"): S_t.T, z.T, h.T are
[64 feat, 512 nodes]; zero on-chip transposes. Steps processed in pairs.

v2 changes vs the 361us baseline:
 - xh matmuls stacked-K: stationary [xt_s0; xt_s1] (K=128 = 2 steps x 64
   feat), moving block-diag [fcw 0; 0 fcw] [128,128] -> one N=128 matmul
   per 128-node tile covers BOTH steps (32 MMs/pair vs 64 LDW-bound N=64).
 - S matmul in fp8 e4m3 with perf_mode=DoubleRow (2 K-tiles per MM,
   2 mults/cell/cycle): 16 DR MMs/pair instead of 32 bf16. adj scaled
   x4096 and xh x16 to sit in e4m3 range; 1/65536 folded into Mz/Mr/Mh
   (exact powers of 2).
 - x streamed as fp8 (halves DMA), relu emits fp8 xh via DVE fused
   (0.5*ps) max 0 tensor_scalar (scale de-folds the fcw x32 fp8 scaling
   and applies the x16 xh range scale).
 - GRU elementwise rebalanced: ZC = 1-z comes free from ACT as
   sigmoid(-ps - bz); combine is h' = h + ZC*(HT - h) (3 ops) + h*r, all
   on GPSIMD (SBUF-only engine, otherwise idle); PSUM->SBUF copies split
   ACT/DVE.

Dtypes: S-path fp8 e4m3 (TRN variant, max 240 — values stay < 50);
gates/h bf16; PSUM f32. Everything rides on errors averaging out across
the 4096-term positive-weighted S sums (~0.05% rel on S).
"""

import os
import sys

sys.path.insert(0, "/opt/trn_rl_repo")

import numpy as np
import ml_dtypes

T, N, F_IN, H1, F_OUT = 48, 4096, 64, 64, 64
NCORES = 8
NS = N // NCORES          # nodes per core = 512
PAIRS = T // 2            # 24
KT = N // 128             # 32 node k-tiles
ADJ_SCALE = 4096.0        # adj -> e4m3 range
XH_SCALE = 16.0           # xh -> e4m3 range
FCW_SCALE = 32.0          # fcw -> e4m3 range

_cache = {}


def _build():
    import concourse.bass as bass
    import concourse.mybir as mybir
    import concourse.tile as tile
    from concourse import bacc

    f32 = mybir.dt.float32
    bf16 = mybir.dt.bfloat16
    fp8 = mybir.dt.float8e4
    AF = mybir.ActivationFunctionType
    ALU = mybir.AluOpType
    DR = mybir.MatmulPerfMode.DoubleRow

    nc = bacc.Bacc(
        "TRN2",
        target_bir_lowering=False,
        debug=False,
        enable_asserts=False,
        num_devices=NCORES,
    )

    # DRAM parameters (per-core shapes)
    adjT_d = nc.dram_tensor("adjT", [128, KT, NS], fp8, kind="ExternalInput").ap()
    xT_d = nc.dram_tensor("xT", [PAIRS, 128, N], fp8, kind="ExternalInput").ap()
    fcw_d = nc.dram_tensor("fcw", [128, 128], fp8, kind="ExternalInput").ap()
    wzr_d = nc.dram_tensor("wzr", [128, 128], bf16, kind="ExternalInput").ap()
    wh_d = nc.dram_tensor("wh", [128, F_OUT], bf16, kind="ExternalInput").ap()
    bz_d = nc.dram_tensor("bz", [F_OUT, 1], f32, kind="ExternalInput").ap()
    bzn_d = nc.dram_tensor("bzn", [F_OUT, 1], f32, kind="ExternalInput").ap()
    br_d = nc.dram_tensor("br", [F_OUT, 1], f32, kind="ExternalInput").ap()
    bh_d = nc.dram_tensor("bh", [F_OUT, 1], f32, kind="ExternalInput").ap()
    out_d = nc.dram_tensor("out", [F_OUT, NS], f32, kind="ExternalOutput").ap()

    with tile.TileContext(nc) as tc:
        with (
            tc.tile_pool(name="const", bufs=1) as constp,
            tc.tile_pool(name="state", bufs=1) as statep,
            tc.tile_pool(name="xt", bufs=2) as xtp,
            tc.tile_pool(name="xh", bufs=2) as xhp,
            tc.tile_pool(name="gw", bufs=3) as gwp,
            tc.tile_pool(name="psx", bufs=2, space="PSUM") as psxp,
            tc.tile_pool(name="pss", bufs=2, space="PSUM") as pssp,
            tc.tile_pool(name="pszr", bufs=1, space="PSUM") as pszrp,
            tc.tile_pool(name="psh", bufs=1, space="PSUM") as pshp,
        ):
            # ---- constants ----
            fcw_sb = constp.tile([128, 128], fp8)
            nc.sync.dma_start(out=fcw_sb[:], in_=fcw_d[:])
            adjT_sb = constp.tile([128, KT, NS], fp8)
            for q, eng in enumerate((nc.sync, nc.gpsimd, nc.gpsimd, nc.gpsimd)):
                eng.dma_start(
                    out=adjT_sb[:, q * 8 : (q + 1) * 8, :],
                    in_=adjT_d[:, q * 8 : (q + 1) * 8, :],
                )
            wzr_sb = constp.tile([128, 128], bf16)
            wh_sb = constp.tile([128, F_OUT], bf16)
            bz_sb = constp.tile([F_OUT, 1], f32)
            bzn_sb = constp.tile([F_OUT, 1], f32)
            br_sb = constp.tile([F_OUT, 1], f32)
            bh_sb = constp.tile([F_OUT, 1], f32)
            for dst, src in (
                (wzr_sb, wzr_d), (wh_sb, wh_d),
                (bz_sb, bz_d), (bzn_sb, bzn_d), (br_sb, br_d), (bh_sb, bh_d),
            ):
                nc.gpsimd.dma_start(out=dst[:], in_=src[:])

            # ---- state ----
            # Concat rhs tiles for the K=128 gate matmuls: rows 0-63 carry
            # S_t.T (refreshed per pair, off-chain), rows 64-127 the
            # recurrent state: h.T in CzS_*, (h*r).T in ChS_*.
            CzS = []
            ChS = []
            for i in range(4):
                czsi = statep.tile([128, NS], bf16, tag=f"CzS{i}")
                chsi = statep.tile([128, NS], bf16, tag=f"ChS{i}")
                CzS.append(czsi)
                ChS.append(chsi)
            nc.vector.memset(CzS[0][:], 0.0)

            def emit_xh_group(xt, xh, g):
                # one stacked-K matmul per 128-node tile: out[128 nodes,
                # 128 = s0 feats | s1 feats]; 8 node tiles fill 2 PSUM banks
                ps = psxp.tile([128, 1024], mybir.dt.float32)
                for j in range(8):
                    k = 8 * g + j
                    nc.tensor.matmul(
                        ps[:, j * 128 : (j + 1) * 128],
                        lhsT=xt[:, k * 128 : (k + 1) * 128],
                        rhs=fcw_sb[:],
                        start=True, stop=True,
                    )
                # xh16 = max((XH/FCW)*ps, 0) -> fp8; DVE fused mult+max
                nc.vector.tensor_scalar(
                    xh[:, 8 * g : 8 * (g + 1), :].rearrange("p a b -> p (a b)"),
                    ps[:], XH_SCALE / FCW_SCALE, 0.0, ALU.mult, ALU.max,
                )

            def emit_gru_front(step):
                # zr matmul + sigmoids + h*r for one step
                cur = CzS[step % 4]
                ch = ChS[step % 4]
                H = cur[64:128, :]

                ps_zr = pszrp.tile([128, NS], mybir.dt.float32, tag="ps_zr")
                nc.tensor.matmul(ps_zr[:], lhsT=wzr_sb[:], rhs=cur[:],
                                 start=True, stop=True)
                # r first: it gates the h-candidate matmul (critical chain)
                R = gwp.tile([128, NS], bf16, tag="R")
                nc.scalar.activation(R[64:128, :], ps_zr[64:128, :],
                                     AF.Sigmoid, bias=br_sb[:])
                # ZC = 1-z = sigmoid(-(ps+bz)) straight from ACT
                ZC = gwp.tile([128, NS], bf16, tag="ZC")
                nc.scalar.activation(ZC[64:128, :], ps_zr[0:64, :],
                                     AF.Sigmoid, bias=bzn_sb[:], scale=-1.0)
                nc.gpsimd.tensor_mul(ch[64:128, :], H, R[64:128, :])
                return ZC

            def emit_gru_back(step, ZC):
                # h-candidate matmul + tanh + combine into the next buffer:
                # h' = h + ZC*(HT - h)
                cur = CzS[step % 4]
                ch = ChS[step % 4]
                nxt = CzS[(step + 1) % 4]
                H = cur[64:128, :]
                ps_h = pshp.tile([F_OUT, NS], mybir.dt.float32)
                nc.tensor.matmul(ps_h[:], lhsT=wh_sb[:], rhs=ch[:],
                                 start=True, stop=True)
                HT = gwp.tile([128, NS], bf16, tag="HT")
                nc.scalar.activation(HT[64:128, :], ps_h[:], AF.Tanh,
                                     bias=bh_sb[:])
                D = gwp.tile([128, NS], bf16, tag="D")
                nc.gpsimd.tensor_sub(D[64:128, :], HT[64:128, :], H)
                E = gwp.tile([128, NS], bf16, tag="E")
                nc.gpsimd.tensor_mul(E[64:128, :], ZC[64:128, :], D[64:128, :])
                nc.gpsimd.tensor_add(nxt[64:128, :], H, E[64:128, :])

            # ---- main loop, software-pipelined: gates of pair p-1 are
            # emitted between the xh matmul groups of pair p ----
            for p in range(PAIRS):
                xt = xtp.tile([128, N], fp8)
                (nc.scalar if p % 2 == 0 else nc.sync).dma_start(
                    out=xt[:], in_=xT_d[p]
                )
                xh = xhp.tile([128, KT, 128], fp8)

                if p >= 1:
                    zc0 = emit_gru_front(2 * p - 2)
                emit_xh_group(xt, xh, 0)
                if p >= 1:
                    emit_gru_back(2 * p - 2, zc0)
                emit_xh_group(xt, xh, 1)
                if p >= 1:
                    zc1 = emit_gru_front(2 * p - 1)
                emit_xh_group(xt, xh, 2)
                if p >= 1:
                    emit_gru_back(2 * p - 1, zc1)
                emit_xh_group(xt, xh, 3)

                # S-pair matmul: psS[2*64 feat, 512 my-nodes] accumulated
                # over 16 fp8 DoubleRow k-tile pairs
                psS = pssp.tile([128, NS], mybir.dt.float32)
                for k in range(KT // 2):
                    nc.tensor.matmul(
                        psS[:],
                        lhsT=xh[:, 2 * k : 2 * k + 2, :],
                        rhs=adjT_sb[:, 2 * k : 2 * k + 2, :],
                        start=(k == 0), stop=(k == KT // 2 - 1),
                        perf_mode=DR,
                    )
                # refresh concat tops for this pair's two steps (buffers
                # last read two pairs ago -> fully off the gate chain)
                s0, s1 = (2 * p) % 4, (2 * p + 1) % 4
                nc.scalar.copy(CzS[s0][0:64, :], psS[0:64, :])
                nc.vector.tensor_copy(ChS[s0][0:64, :], psS[0:64, :])
                nc.scalar.copy(CzS[s1][0:64, :], psS[64:128, :])
                nc.vector.tensor_copy(ChS[s1][0:64, :], psS[64:128, :])

            # drain: gates for the last pair
            zc = emit_gru_front(2 * PAIRS - 2)
            emit_gru_back(2 * PAIRS - 2, zc)
            zc = emit_gru_front(2 * PAIRS - 1)
            emit_gru_back(2 * PAIRS - 1, zc)

            Hout = statep.tile([F_OUT, NS], f32)
            nc.scalar.copy(Hout[:], CzS[(2 * PAIRS) % 4][64:128, :])
            nc.sync.dma_start(out=out_d[:], in_=Hout[:])

    nc.compile()
    return nc


def _prep_inputs(x, adj, fc_w, Wz, Wr, Wh, Lz, Lr, Lh, bz, br, bh):
    bf16 = ml_dtypes.bfloat16
    fp8 = ml_dtypes.float8_e4m3
    f32 = np.float32

    # x [T, N, F] -> [PAIRS, 2*F, N]: partition dim = (step, feat)
    xT = np.ascontiguousarray(
        x.reshape(PAIRS, 2, N, F_IN).transpose(0, 1, 3, 2).reshape(PAIRS, 128, N)
    ).astype(fp8)
    # block-diag [fcw 0; 0 fcw] so one matmul covers both steps
    fcw_stack = np.zeros((128, 128), f32)
    fcw_stack[0:64, 0:64] = fc_w * FCW_SCALE
    fcw_stack[64:128, 64:128] = fc_w * FCW_SCALE
    fcw_stack = fcw_stack.astype(fp8)

    gate_scale = 1.0 / (ADJ_SCALE * XH_SCALE)  # S arrives x65536

    def fold(W, L):
        m = W.astype(np.float64) @ L[:F_OUT].astype(np.float64)
        return (m * gate_scale).astype(bf16)

    mz, mr, mh = fold(Wz, Lz), fold(Wr, Lr), fold(Wh, Lh)
    mzr = np.concatenate([mz, mr], axis=1)  # [64, 128]: z cols | r cols
    lzr = np.concatenate(
        [Lz[F_OUT:].astype(bf16), Lr[F_OUT:].astype(bf16)], axis=1
    )
    wzr = np.concatenate([mzr, lzr], axis=0)  # [128, 128]
    wh = np.concatenate([mh, Lh[F_OUT:].astype(bf16)], axis=0)  # [128, 64]
    shared = {
        "xT": xT, "fcw": fcw_stack, "wzr": wzr, "wh": wh,
        "bz": bz.reshape(F_OUT, 1).astype(f32),
        "bzn": (-bz).reshape(F_OUT, 1).astype(f32),
        "br": br.reshape(F_OUT, 1).astype(f32),
        "bh": bh.reshape(F_OUT, 1).astype(f32),
    }
    in_maps = []
    for c in range(NCORES):
        m = dict(shared)
        at = adj[c * NS : (c + 1) * NS, :].T * ADJ_SCALE  # [N, NS]
        m["adjT"] = np.ascontiguousarray(
            at.reshape(KT, 128, NS).transpose(1, 0, 2)
        ).astype(fp8)
        in_maps.append(m)
    return in_maps


def kernel(x, adj, fc_w, fc_b, Wz, Wr, Wh, Lz, Lr, Lh, bz, br, bh):
    x = np.asarray(x, np.float32)
    adj = np.asarray(adj, np.float32)
    args = [np.asarray(a, np.float32) for a in (fc_w, Wz, Wr, Wh, Lz, Lr, Lh, bz, br, bh)]
    fc_b = np.asarray(fc_b, np.float32)
    if np.any(fc_b != 0.0):
        # fc_b can't fold into the per-partition activation bias (it varies
        # along the free dim); the reference always passes zeros. Pure-numpy
        # fallback keeps kernel() correct for arbitrary inputs.
        return _numpy_ref(x, adj, args[0], fc_b, *args[1:])

    from concourse.bass_utils import run_bass_kernel_spmd

    if "nc" not in _cache:
        _cache["nc"] = _build()
    nc = _cache["nc"]

    in_maps = _prep_inputs(x, adj, *args)
    trace = bool(int(os.environ.get("BASS_KERNEL_TRACE", "0")))
    kwargs = {}
    if trace:
        _install_trace_shim()
        tmpdir = os.environ.get("BASS_KERNEL_TRACE_DIR")
        if tmpdir:
            os.makedirs(tmpdir, exist_ok=True)
            kwargs["tmpdir"] = tmpdir
    res = run_bass_kernel_spmd(
        nc, in_maps, core_ids=list(range(NCORES)), trace=trace, **kwargs
    )
    _cache["last_result"] = res

    out = np.empty((1, N, F_OUT), np.float32)
    for c in range(NCORES):
        out[0, c * NS : (c + 1) * NS, :] = res.results[c]["out"].T
    return out


def _install_trace_shim():
    """Register the NTFF profile hook (this image's antenv lacks axon_hooks)
    and stub out the artifact upload so profiling works offline."""
    import types

    try:
        from antenv import axon_hooks  # noqa: F401
        return
    except ImportError:
        pass
    sys.path.insert(0, "/root/.axon_site")
    from trn_agent_boot.trn_boot import _ntff_profile_via_ctypes

    hook = _ntff_profile_via_ctypes("/opt/axon/libaxon_pjrt.so")
    m = types.ModuleType("antenv.axon_hooks")
    m.get_axon_ntff_profile_hook = lambda: hook
    m.set_axon_ntff_profile_hook = lambda h: None
    sys.modules["antenv.axon_hooks"] = m
    import antenv

    antenv.axon_hooks = m
    from concourse import bass_utils as _bu

    _bu.upload_artifacts = lambda tmpdir: tmpdir


def _numpy_ref(x, adj, fc_w, fc_b, Wz, Wr, Wh, Lz, Lr, Lh, bz, br, bh):
    def sigmoid(v):
        return 1.0 / (1.0 + np.exp(-v))

    xh = np.maximum(x @ fc_w + fc_b, 0.0)
    h = np.zeros((N, F_OUT), np.float32)
    for t in range(T):
        s = adj @ xh[t]
        az, ar, ah = s @ Wz, s @ Wr, s @ Wh
        z = sigmoid(np.concatenate([az, h], -1) @ Lz + bz)
        r = sigmoid(np.concatenate([ar, h], -1) @ Lr + br)
        ht = np.tanh(np.concatenate([ah, h * r], -1) @ Lh + bh)
        h = z * h + (1.0 - z) * ht
    return h[None].astype(np.float32)
